# revision 12
# baseline (speedup 1.0000x reference)
"""InnerAttention kernel for 8 Trainium2 NeuronCores.

Computes, per batch b:
    e = x[b] @ y[b].T          [M, N]
    p = softmax(e, axis=-1)    (over n)
    out[b] = p.T @ x[b]        [N, D]

Sharding: data-parallel over batch (B=8 -> one batch per core). Full inputs in,
full output out; per-core slices are shipped via run_bass_kernel_spmd in_maps.

Per-core algorithm (M=N=2048, D=1024, P=128):
  phase 0: y -> yT (fp32r, d-major, per-n-slice tiles) via PE transposes
  phase 1: per m-tile (16):
    x m-tile -> xT (fp32r, d-major) via PE transposes
    mm1: e[128, 2048] in PSUM, single fp32r pass (1 cycle/row at free=512)
    softmax: DVE row-max over PSUM, ACT exp (bias=-max) -> p bf16 in SBUF,
         accum_out gives row-sum; 1/sum folded into xs = x * (1/s) in bf16
  phase 2: per (n-chunk 128, d-half 512) out tile: accumulate all 16
    p.T @ xs contributions (bf16) in one PSUM bank, stage, single DMA out.
"""

import numpy as np

import concourse.bacc as bacc
import concourse.mybir as mybir
import concourse.tile as tile
from concourse import bass_utils

B, M, N, D = 8, 2048, 2048, 1024
P = 128
NSLICE = 512          # matmul moving free-dim (one PSUM bank of fp32)
N_MTILES = M // P     # 16
N_DCHUNK = D // P     # 8
N_NSL = N // NSLICE   # 4
N_NCHUNK = N // P     # 16
N_DHALF = D // NSLICE  # 2

F32 = mybir.dt.float32
F32R = mybir.dt.float32r
BF16 = mybir.dt.bfloat16
FP16 = mybir.dt.float16
AX = mybir.AxisListType.X
EXP = mybir.ActivationFunctionType.Exp


def _build_nc():
    nc = bacc.Bacc("TRN2", target_bir_lowering=False, debug=False)
    x_d = nc.dram_tensor("x", [M, D], F32R, kind="ExternalInput").ap()
    y_d = nc.dram_tensor("y", [N, D], F32R, kind="ExternalInput").ap()
    ident_d = nc.dram_tensor("ident", [P, P], F32R, kind="ExternalInput").ap()
    out_d = nc.dram_tensor("out", [N, D], F32, kind="ExternalOutput").ap()

    with tile.TileContext(nc) as tc:
        with (
            tc.tile_pool(name="const", bufs=1) as constp,
            tc.tile_pool(name="yT", bufs=1) as yTp,
            tc.tile_pool(name="pP", bufs=1) as pPp,
            tc.tile_pool(name="xsP", bufs=1) as xsPp,
            tc.tile_pool(name="work", bufs=2) as work,
            tc.tile_pool(name="stats", bufs=3) as stats,
        ):
            # identity for PE transposes (fp32r: 1.5 cycles/row vs 2.0 for f32)
            ident = constp.tile([P, P], F32R, tag="id32")
            nc.sync.dma_start(ident[:], ident_d)

            # persistent yT: per n-slice tiles [128 d, 8 chunks x 512 n]
            # fp32r; chunk k occupies columns [k*512, (k+1)*512)
            yT = [yTp.tile([P, N_DCHUNK * NSLICE], FP16, tag=f"yT{s}", name=f"yT{s}")
                  for s in range(N_NSL)]
            # persistent p (bf16) and scaled-x (bf16) for mm2
            pT = [pPp.tile([P, N], BF16, tag=f"p{mi}", name=f"p{mi}")
                  for mi in range(N_MTILES)]
            xs = [xsPp.tile([P, D], BF16, tag=f"xs{mi}", name=f"xs{mi}")
                  for mi in range(N_MTILES)]

            with (
                tc.tile_pool(name="tps", bufs=2, space="PSUM") as tpsp,
                tc.tile_pool(name="eps", bufs=6, space="PSUM") as epsp,
            ):
                def transpose_split(src, sink):
                    """PE-transpose fp32r [128, 1024] in two 4-chunk batches;
                    sink consumes (half, psum [128, 4x128] fp32r)."""
                    for half in range(2):
                        ps = tpsp.tile([P, NSLICE], F32R, tag="tp", name="tp")
                        for j in range(4):
                            k = half * 4 + j
                            nc.tensor.transpose(
                                ps[:, j * P:(j + 1) * P],
                                src[:, k * P:(k + 1) * P],
                                ident[:],
                            )
                        sink(half, ps)

                # x m-tile load + PE-transpose, software-pipelined one tile
                # ahead of mm1 so the DVE psum->xT copies hide under mm1
                # streams of the previous tile.
                xT_of = {}
                xnat_of = {}

                def emit_xpose(mi):
                    xnat = work.tile([P, D], F32R, tag="xnat", bufs=3)
                    nc.sync.dma_start(xnat[:], x_d[mi * P:(mi + 1) * P, :])
                    xTh = [work.tile([P, NSLICE], FP16, tag=f"xT{h}", name="xT")
                           for h in range(2)]

                    def xsink(half, ps):
                        nc.vector.tensor_copy(xTh[half][:], ps[:])
                    transpose_split(xnat, xsink)
                    xT_of[mi] = xTh
                    xnat_of[mi] = xnat

                def emit_mm1_group(mi, ns):
                    """one n-slice of e for m-tile mi -> psum bank (fp32r)"""
                    xTh = xT_of[mi]
                    ep = epsp.tile([P, NSLICE], F32, tag="e", name="eps")
                    for k in range(N_DCHUNK):
                        nc.tensor.matmul(
                            ep[:], xTh[k // 4][:, (k % 4) * P:(k % 4 + 1) * P],
                            yT[ns][:, k * NSLICE:(k + 1) * NSLICE],
                            start=(k == 0), stop=(k == N_DCHUNK - 1),
                        )
                    return ep

                def emit_softmax(mi, eps):
                    rmax4 = stats.tile([P, N_NSL], F32, tag="rmax4")
                    for ns in range(N_NSL):
                        nc.vector.reduce_max(rmax4[:, ns:ns + 1], eps[ns][:], axis=AX)
                    negmax = stats.tile([P, 1], F32, tag="negmax")
                    rmax = stats.tile([P, 1], F32, tag="rmax")
                    nc.vector.reduce_max(rmax[:], rmax4[:], axis=AX)
                    nc.vector.tensor_scalar_mul(negmax[:], rmax[:], -1.0)

                    s4 = stats.tile([P, N_NSL], F32, tag="s4")
                    for ns in range(N_NSL):
                        c0 = ns * NSLICE
                        nc.scalar.activation(
                            pT[mi][:, c0:c0 + NSLICE], eps[ns][:], EXP,
                            bias=negmax[:], accum_out=s4[:, ns:ns + 1],
                        )
                    ssum = stats.tile([P, 1], F32, tag="ssum")
                    nc.vector.reduce_sum(ssum[:], s4[:], axis=AX)
                    rinv = stats.tile([P, 1], F32, tag="rinv")
                    nc.vector.reciprocal(rinv[:], ssum[:])
                    nc.vector.tensor_scalar_mul(xs[mi][:], xnat_of.pop(mi)[:], rinv[:])
                    del xT_of[mi]

                # ---- phase 0 (y -> yT) fused with tile-0 mm1: tile 0's
                # n-slice group ns only needs yT[ns], so it slots into the
                # DMA-gated idle after y-slice ns is transposed. ----
                eps0 = []
                for sl in range(N_NSL):
                    for i in range(4 * sl, 4 * sl + 4):
                        c0 = (i % 4) * P
                        ynat = work.tile([P, D], F32R, tag="ynat", bufs=4)
                        nc.sync.dma_start(ynat[:], y_d[i * P:(i + 1) * P, :])

                        def ysink(half, ps, sl=sl, c0=c0):
                            dst = yT[sl].rearrange("p (k c) -> p k c", c=NSLICE)[
                                :, half * 4:half * 4 + 4, c0:c0 + P]
                            src = ps.rearrange("p (j c) -> p j c", c=P)
                            nc.vector.tensor_copy(dst, src)
                        transpose_split(ynat, ysink)
                    if sl == 0:
                        emit_xpose(0)
                    eps0.append(emit_mm1_group(0, sl))

                # ---- phase 1 steady state ----
                emit_xpose(1)
                emit_softmax(0, eps0)
                for mi in range(1, N_MTILES):
                    eps = []
                    for ns in range(N_NSL):
                        eps.append(emit_mm1_group(mi, ns))
                        if ns == 0 and mi + 1 < N_MTILES:
                            emit_xpose(mi + 1)
                    emit_softmax(mi, eps)

            # ---- phase 2: out[nch, dh] = sum_mi p[mi].T @ xs[mi], one flush ----
            with tc.tile_pool(name="ops", bufs=8, space="PSUM") as opsp:
                for nch in range(N_NCHUNK):
                    for dh in range(N_DHALF):
                        ops = opsp.tile([P, NSLICE], F32, tag="o", name="ops")
                        for mi in range(N_MTILES):
                            nc.tensor.matmul(
                                ops[:],
                                pT[mi][:, nch * P:(nch + 1) * P],
                                xs[mi][:, dh * NSLICE:(dh + 1) * NSLICE],
                                start=(mi == 0), stop=(mi == N_MTILES - 1),
                            )
                        dst = out_d[nch * P:(nch + 1) * P,
                                    dh * NSLICE:(dh + 1) * NSLICE]
                        stage = work.tile([P, NSLICE], F32, tag="ostage", bufs=4)
                        nc.vector.tensor_copy(stage[:], ops[:])
                        nc.sync.dma_start(dst, stage[:])

    nc.compile()
    return nc


_NC_CACHE = {}


def _get_nc():
    if "nc" not in _NC_CACHE:
        _NC_CACHE["nc"] = _build_nc()
    return _NC_CACHE["nc"]


def kernel(x: np.ndarray, y: np.ndarray) -> np.ndarray:
    assert x.shape == (B, M, D) and y.shape == (B, N, D)
    nc = _get_nc()
    ident = np.eye(P, dtype=np.float32)
    in_maps = [
        {
            "x": np.ascontiguousarray(x[b], dtype=np.float32),
            "y": np.ascontiguousarray(y[b], dtype=np.float32),
            "ident": ident,
        }
        for b in range(B)
    ]
    res = bass_utils.run_bass_kernel_spmd(nc, in_maps, core_ids=list(range(B)))
    return np.stack([res.results[b]["out"] for b in range(B)], axis=0)


# revision 16
# speedup vs baseline: 1.0695x; 1.0695x over previous
"""InnerAttention kernel for 8 Trainium2 NeuronCores.

Computes, per batch b:
    e = x[b] @ y[b].T          [M, N]
    p = softmax(e, axis=-1)    (over n)
    out[b] = p.T @ x[b]        [N, D]

Sharding: data-parallel over batch (B=8 -> one batch per core). Full inputs in,
full output out; per-core slices are shipped via run_bass_kernel_spmd in_maps.

Per-core algorithm (M=N=2048, D=1024, P=128):
  phase 0: y -> yT (fp32r, d-major, per-n-slice tiles) via PE transposes
  phase 1: per m-tile (16):
    x m-tile -> xT (fp32r, d-major) via PE transposes
    mm1: e[128, 2048] in PSUM, single fp32r pass (1 cycle/row at free=512)
    softmax: DVE row-max over PSUM, ACT exp (bias=-max) -> p bf16 in SBUF,
         accum_out gives row-sum; 1/sum folded into xs = x * (1/s) in bf16
  phase 2: per (n-chunk 128, d-half 512) out tile: accumulate all 16
    p.T @ xs contributions (bf16) in one PSUM bank, stage, single DMA out.
"""

import numpy as np

import concourse.bacc as bacc
import concourse.mybir as mybir
import concourse.tile as tile
from concourse import bass_utils

B, M, N, D = 8, 2048, 2048, 1024
P = 128
NSLICE = 512          # matmul moving free-dim (one PSUM bank of fp32)
N_MTILES = M // P     # 16
N_DCHUNK = D // P     # 8
N_NSL = N // NSLICE   # 4
N_NCHUNK = N // P     # 16
N_DHALF = D // NSLICE  # 2

F32 = mybir.dt.float32
F32R = mybir.dt.float32r
BF16 = mybir.dt.bfloat16
FP16 = mybir.dt.float16
AX = mybir.AxisListType.X
EXP = mybir.ActivationFunctionType.Exp


def _build_nc():
    nc = bacc.Bacc("TRN2", target_bir_lowering=False, debug=False)
    x_d = nc.dram_tensor("x", [M, D], F32R, kind="ExternalInput").ap()
    y_d = nc.dram_tensor("y", [N, D], F32R, kind="ExternalInput").ap()
    ident_d = nc.dram_tensor("ident", [P, P], F32R, kind="ExternalInput").ap()
    out_d = nc.dram_tensor("out", [N, D], F32, kind="ExternalOutput").ap()

    with tile.TileContext(nc) as tc:
        with (
            tc.tile_pool(name="const", bufs=1) as constp,
            tc.tile_pool(name="yT", bufs=1) as yTp,
            tc.tile_pool(name="pP", bufs=1) as pPp,
            tc.tile_pool(name="xsP", bufs=1) as xsPp,
            tc.tile_pool(name="work", bufs=2) as work,
            tc.tile_pool(name="stats", bufs=3) as stats,
        ):
            # identity for PE transposes (fp32r: 1.5 cycles/row vs 2.0 for f32)
            ident = constp.tile([P, P], F32R, tag="id32")
            nc.sync.dma_start(ident[:], ident_d)

            # persistent yT: per n-slice tiles [128 d, 8 chunks x 512 n]
            # fp32r; chunk k occupies columns [k*512, (k+1)*512)
            yT = [yTp.tile([P, N_DCHUNK * NSLICE], FP16, tag=f"yT{s}", name=f"yT{s}")
                  for s in range(N_NSL)]
            # persistent p (bf16) and scaled-x (bf16) for mm2
            pT = [pPp.tile([P, N], BF16, tag=f"p{mi}", name=f"p{mi}")
                  for mi in range(N_MTILES)]
            xs = [xsPp.tile([P, D], BF16, tag=f"xs{mi}", name=f"xs{mi}")
                  for mi in range(N_MTILES)]

            with (
                tc.tile_pool(name="tps", bufs=2, space="PSUM") as tpsp,
                tc.tile_pool(name="eps", bufs=6, space="PSUM") as epsp,
            ):
                def transpose_split(src, sink):
                    """PE-transpose fp32r [128, 1024] in two 4-chunk batches;
                    sink consumes (half, psum [128, 4x128] fp32r)."""
                    for half in range(2):
                        ps = tpsp.tile([P, NSLICE], F32R, tag="tp", name="tp")
                        for j in range(4):
                            k = half * 4 + j
                            nc.tensor.transpose(
                                ps[:, j * P:(j + 1) * P],
                                src[:, k * P:(k + 1) * P],
                                ident[:],
                            )
                        sink(half, ps)

                # x m-tile load + PE-transpose, software-pipelined one tile
                # ahead of mm1 so the DVE psum->xT copies hide under mm1
                # streams of the previous tile.
                xT_of = {}
                xnat_of = {}

                def emit_xdma(mi):
                    if mi >= N_MTILES:
                        return
                    xnat = work.tile([P, D], F32R, tag="xnat", bufs=4)
                    nc.sync.dma_start(xnat[:], x_d[mi * P:(mi + 1) * P, :])
                    xnat_of[mi] = xnat

                def emit_xpose(mi):
                    xTh = [work.tile([P, NSLICE], FP16, tag=f"xT{h}", name="xT")
                           for h in range(2)]

                    def xsink(half, ps):
                        nc.vector.tensor_copy(xTh[half][:], ps[:])
                    transpose_split(xnat_of[mi], xsink)
                    xT_of[mi] = xTh

                def emit_mm1_group(mi, ns):
                    """one n-slice of e for m-tile mi -> psum bank (fp32r)"""
                    xTh = xT_of[mi]
                    ep = epsp.tile([P, NSLICE], F32, tag="e", name="eps")
                    for k in range(N_DCHUNK):
                        nc.tensor.matmul(
                            ep[:], xTh[k // 4][:, (k % 4) * P:(k % 4 + 1) * P],
                            yT[ns][:, k * NSLICE:(k + 1) * NSLICE],
                            start=(k == 0), stop=(k == N_DCHUNK - 1),
                        )
                    return ep

                def emit_softmax(mi, eps):
                    rmax4 = stats.tile([P, N_NSL], F32, tag="rmax4")
                    for ns in range(N_NSL):
                        nc.vector.reduce_max(rmax4[:, ns:ns + 1], eps[ns][:], axis=AX)
                    negmax = stats.tile([P, 1], F32, tag="negmax")
                    rmax = stats.tile([P, 1], F32, tag="rmax")
                    nc.vector.reduce_max(rmax[:], rmax4[:], axis=AX)
                    nc.vector.tensor_scalar_mul(negmax[:], rmax[:], -1.0)

                    s4 = stats.tile([P, N_NSL], F32, tag="s4")
                    for ns in range(N_NSL):
                        c0 = ns * NSLICE
                        nc.scalar.activation(
                            pT[mi][:, c0:c0 + NSLICE], eps[ns][:], EXP,
                            bias=negmax[:], accum_out=s4[:, ns:ns + 1],
                        )
                    ssum = stats.tile([P, 1], F32, tag="ssum")
                    nc.vector.reduce_sum(ssum[:], s4[:], axis=AX)
                    rinv = stats.tile([P, 1], F32, tag="rinv")
                    nc.vector.reciprocal(rinv[:], ssum[:])
                    nc.vector.tensor_scalar_mul(xs[mi][:], xnat_of.pop(mi)[:], rinv[:])
                    del xT_of[mi]

                # ---- phase 0 (y -> yT) fused with tile-0 mm1: tile 0's
                # n-slice group ns only needs yT[ns], so it slots into the
                # DMA-gated idle after y-slice ns is transposed. ----
                emit_xdma(0)
                emit_xdma(1)
                eps0 = []
                for sl in range(N_NSL):
                    for i in range(4 * sl, 4 * sl + 4):
                        c0 = (i % 4) * P
                        ynat = work.tile([P, D], F32R, tag="ynat", bufs=4)
                        nc.sync.dma_start(ynat[:], y_d[i * P:(i + 1) * P, :])

                        def ysink(half, ps, sl=sl, c0=c0):
                            dst = yT[sl].rearrange("p (k c) -> p k c", c=NSLICE)[
                                :, half * 4:half * 4 + 4, c0:c0 + P]
                            src = ps.rearrange("p (j c) -> p j c", c=P)
                            nc.vector.tensor_copy(dst, src)
                        transpose_split(ynat, ysink)
                    if sl == 0:
                        emit_xpose(0)
                    eps0.append(emit_mm1_group(0, sl))

                # ---- phase 1 steady state ----
                emit_xdma(2)
                emit_xpose(1)
                emit_softmax(0, eps0)
                for mi in range(1, N_MTILES):
                    eps = []
                    for ns in range(N_NSL):
                        eps.append(emit_mm1_group(mi, ns))
                        if ns == 0:
                            emit_xdma(mi + 2)
                            if mi + 1 < N_MTILES:
                                emit_xpose(mi + 1)
                    emit_softmax(mi, eps)

                # ---- phase 2: out[nch, dh] = sum_mi p[mi].T @ xs[mi] ----
                # same psum pool as mm1 (no pool transition barrier)
                for nch in range(N_NCHUNK):
                    for dh in range(N_DHALF):
                        ops = epsp.tile([P, NSLICE], F32, tag="e", name="ops")
                        for mi in range(N_MTILES):
                            nc.tensor.matmul(
                                ops[:],
                                pT[mi][:, nch * P:(nch + 1) * P],
                                xs[mi][:, dh * NSLICE:(dh + 1) * NSLICE],
                                start=(mi == 0), stop=(mi == N_MTILES - 1),
                            )
                        dst = out_d[nch * P:(nch + 1) * P,
                                    dh * NSLICE:(dh + 1) * NSLICE]
                        stage = work.tile([P, NSLICE], F32, tag="ostage", bufs=4)
                        nc.vector.tensor_copy(stage[:], ops[:])
                        nc.sync.dma_start(dst, stage[:])

    nc.compile()
    return nc


_NC_CACHE = {}


def _get_nc():
    if "nc" not in _NC_CACHE:
        _NC_CACHE["nc"] = _build_nc()
    return _NC_CACHE["nc"]


def kernel(x: np.ndarray, y: np.ndarray) -> np.ndarray:
    assert x.shape == (B, M, D) and y.shape == (B, N, D)
    nc = _get_nc()
    ident = np.eye(P, dtype=np.float32)
    in_maps = [
        {
            "x": np.ascontiguousarray(x[b], dtype=np.float32),
            "y": np.ascontiguousarray(y[b], dtype=np.float32),
            "ident": ident,
        }
        for b in range(B)
    ]
    res = bass_utils.run_bass_kernel_spmd(nc, in_maps, core_ids=list(range(B)))
    return np.stack([res.results[b]["out"] for b in range(B)], axis=0)


# revision 18
# speedup vs baseline: 1.0892x; 1.0184x over previous
"""InnerAttention kernel for 8 Trainium2 NeuronCores.

Computes, per batch b:
    e = x[b] @ y[b].T          [M, N]
    p = softmax(e, axis=-1)    (over n)
    out[b] = p.T @ x[b]        [N, D]

Sharding: data-parallel over batch (B=8 -> one batch per core). Full inputs in,
full output out; per-core slices are shipped via run_bass_kernel_spmd in_maps.

Per-core algorithm (M=N=2048, D=1024, P=128):
  phase 0: y -> yT (fp32r, d-major, per-n-slice tiles) via PE transposes
  phase 1: per m-tile (16):
    x m-tile -> xT (fp32r, d-major) via PE transposes
    mm1: e[128, 2048] in PSUM, single fp32r pass (1 cycle/row at free=512)
    softmax: DVE row-max over PSUM, ACT exp (bias=-max) -> p bf16 in SBUF,
         accum_out gives row-sum; 1/sum folded into xs = x * (1/s) in bf16
  phase 2: per (n-chunk 128, d-half 512) out tile: accumulate all 16
    p.T @ xs contributions (bf16) in one PSUM bank, stage, single DMA out.
"""

import numpy as np

import concourse.bacc as bacc
import concourse.mybir as mybir
import concourse.tile as tile
from concourse import bass_utils

B, M, N, D = 8, 2048, 2048, 1024
P = 128
NSLICE = 512          # matmul moving free-dim (one PSUM bank of fp32)
N_MTILES = M // P     # 16
N_DCHUNK = D // P     # 8
N_NSL = N // NSLICE   # 4
N_NCHUNK = N // P     # 16
N_DHALF = D // NSLICE  # 2

F32 = mybir.dt.float32
F32R = mybir.dt.float32r
BF16 = mybir.dt.bfloat16
FP16 = mybir.dt.float16
AX = mybir.AxisListType.X
EXP = mybir.ActivationFunctionType.Exp
CPY = mybir.ActivationFunctionType.Copy


def _build_nc():
    nc = bacc.Bacc("TRN2", target_bir_lowering=False, debug=False)
    x_d = nc.dram_tensor("x", [M, D], F32R, kind="ExternalInput").ap()
    y_d = nc.dram_tensor("y", [N, D], F32R, kind="ExternalInput").ap()
    ident_d = nc.dram_tensor("ident", [P, P], FP16, kind="ExternalInput").ap()
    out_d = nc.dram_tensor("out", [N, D], F32, kind="ExternalOutput").ap()

    with tile.TileContext(nc) as tc:
        with (
            tc.tile_pool(name="const", bufs=1) as constp,
            tc.tile_pool(name="yT", bufs=1) as yTp,
            tc.tile_pool(name="pP", bufs=1) as pPp,
            tc.tile_pool(name="xsP", bufs=1) as xsPp,
            tc.tile_pool(name="work", bufs=2) as work,
            tc.tile_pool(name="stats", bufs=3) as stats,
        ):
            # identity for PE transposes (fp16: 1 cycle/row)
            ident = constp.tile([P, P], FP16, tag="id16")
            nc.sync.dma_start(ident[:], ident_d)

            # persistent yT: per n-slice tiles [128 d, 8 chunks x 512 n]
            # fp32r; chunk k occupies columns [k*512, (k+1)*512)
            yT = [yTp.tile([P, N_DCHUNK * NSLICE], FP16, tag=f"yT{s}", name=f"yT{s}")
                  for s in range(N_NSL)]
            # persistent p (bf16) and scaled-x (bf16) for mm2
            pT = [pPp.tile([P, N], BF16, tag=f"p{mi}", name=f"p{mi}")
                  for mi in range(N_MTILES)]
            xs = [xsPp.tile([P, D], BF16, tag=f"xs{mi}", name=f"xs{mi}")
                  for mi in range(N_MTILES)]

            with (
                tc.tile_pool(name="tps", bufs=2, space="PSUM") as tpsp,
                tc.tile_pool(name="eps", bufs=6, space="PSUM") as epsp,
            ):
                def transpose_split(src, sink):
                    """PE-transpose fp16 [128, 1024] in two 4-chunk batches;
                    sink consumes (half, psum [128, 4x128] fp16)."""
                    for half in range(2):
                        ps = tpsp.tile([P, NSLICE], FP16, tag="tp", name="tp")
                        for j in range(4):
                            k = half * 4 + j
                            nc.tensor.transpose(
                                ps[:, j * P:(j + 1) * P],
                                src[:, k * P:(k + 1) * P],
                                ident[:],
                            )
                        sink(half, ps)

                # x m-tile load + PE-transpose, software-pipelined one tile
                # ahead of mm1 so the DVE psum->xT copies hide under mm1
                # streams of the previous tile.
                xT_of = {}
                xnat_of = {}

                def emit_xdma(mi):
                    if mi >= N_MTILES:
                        return
                    xnat = work.tile([P, D], F32R, tag="xnat", bufs=4)
                    nc.sync.dma_start(xnat[:], x_d[mi * P:(mi + 1) * P, :])
                    xnat_of[mi] = xnat

                def emit_xpose(mi):
                    xh16 = work.tile([P, D], FP16, tag="xh16", bufs=2)
                    nc.scalar.activation(xh16[:], xnat_of[mi][:], CPY)
                    xTh = [work.tile([P, NSLICE], FP16, tag=f"xT{h}", name="xT")
                           for h in range(2)]

                    def xsink(half, ps):
                        nc.vector.tensor_copy(xTh[half][:], ps[:])
                    transpose_split(xh16, xsink)
                    xT_of[mi] = xTh

                def emit_mm1_group(mi, ns):
                    """one n-slice of e for m-tile mi -> psum bank (fp32r)"""
                    xTh = xT_of[mi]
                    ep = epsp.tile([P, NSLICE], F32, tag="e", name="eps")
                    for k in range(N_DCHUNK):
                        nc.tensor.matmul(
                            ep[:], xTh[k // 4][:, (k % 4) * P:(k % 4 + 1) * P],
                            yT[ns][:, k * NSLICE:(k + 1) * NSLICE],
                            start=(k == 0), stop=(k == N_DCHUNK - 1),
                        )
                    return ep

                def emit_softmax(mi, eps):
                    rmax4 = stats.tile([P, N_NSL], F32, tag="rmax4")
                    for ns in range(N_NSL):
                        nc.vector.reduce_max(rmax4[:, ns:ns + 1], eps[ns][:], axis=AX)
                    negmax = stats.tile([P, 1], F32, tag="negmax")
                    rmax = stats.tile([P, 1], F32, tag="rmax")
                    nc.vector.reduce_max(rmax[:], rmax4[:], axis=AX)
                    nc.vector.tensor_scalar_mul(negmax[:], rmax[:], -1.0)

                    s4 = stats.tile([P, N_NSL], F32, tag="s4")
                    for ns in range(N_NSL):
                        c0 = ns * NSLICE
                        nc.scalar.activation(
                            pT[mi][:, c0:c0 + NSLICE], eps[ns][:], EXP,
                            bias=negmax[:], accum_out=s4[:, ns:ns + 1],
                        )
                    ssum = stats.tile([P, 1], F32, tag="ssum")
                    nc.vector.reduce_sum(ssum[:], s4[:], axis=AX)
                    rinv = stats.tile([P, 1], F32, tag="rinv")
                    nc.vector.reciprocal(rinv[:], ssum[:])
                    nc.vector.tensor_scalar_mul(xs[mi][:], xnat_of.pop(mi)[:], rinv[:])
                    del xT_of[mi]

                # ---- phase 0 (y -> yT) fused with tile-0 mm1: tile 0's
                # n-slice group ns only needs yT[ns], so it slots into the
                # DMA-gated idle after y-slice ns is transposed. ----
                emit_xdma(0)
                emit_xdma(1)
                eps0 = []
                for sl in range(N_NSL):
                    for i in range(4 * sl, 4 * sl + 4):
                        c0 = (i % 4) * P
                        ynat = work.tile([P, D], F32R, tag="ynat", bufs=4)
                        nc.sync.dma_start(ynat[:], y_d[i * P:(i + 1) * P, :])
                        yh16 = work.tile([P, D], FP16, tag="yh16", bufs=3)
                        nc.scalar.activation(yh16[:], ynat[:], CPY)

                        def ysink(half, ps, sl=sl, c0=c0):
                            dst = yT[sl].rearrange("p (k c) -> p k c", c=NSLICE)[
                                :, half * 4:half * 4 + 4, c0:c0 + P]
                            src = ps.rearrange("p (j c) -> p j c", c=P)
                            nc.vector.tensor_copy(dst, src)
                        transpose_split(yh16, ysink)
                    if sl == 0:
                        emit_xpose(0)
                    eps0.append(emit_mm1_group(0, sl))

                # ---- phase 1 steady state ----
                emit_xdma(2)
                emit_xpose(1)
                emit_softmax(0, eps0)
                for mi in range(1, N_MTILES):
                    eps = []
                    for ns in range(N_NSL):
                        eps.append(emit_mm1_group(mi, ns))
                        if ns == 0:
                            emit_xdma(mi + 2)
                            if mi + 1 < N_MTILES:
                                emit_xpose(mi + 1)
                    emit_softmax(mi, eps)

                # ---- phase 2: out[nch, dh] = sum_mi p[mi].T @ xs[mi] ----
                # same psum pool as mm1 (no pool transition barrier)
                for nch in range(N_NCHUNK):
                    for dh in range(N_DHALF):
                        ops = epsp.tile([P, NSLICE], F32, tag="e", name="ops")
                        for mi in range(N_MTILES):
                            nc.tensor.matmul(
                                ops[:],
                                pT[mi][:, nch * P:(nch + 1) * P],
                                xs[mi][:, dh * NSLICE:(dh + 1) * NSLICE],
                                start=(mi == 0), stop=(mi == N_MTILES - 1),
                            )
                        dst = out_d[nch * P:(nch + 1) * P,
                                    dh * NSLICE:(dh + 1) * NSLICE]
                        stage = work.tile([P, NSLICE], F32, tag="ostage", bufs=4)
                        nc.vector.tensor_copy(stage[:], ops[:])
                        nc.sync.dma_start(dst, stage[:])

    nc.compile()
    return nc


_NC_CACHE = {}


def _get_nc():
    if "nc" not in _NC_CACHE:
        _NC_CACHE["nc"] = _build_nc()
    return _NC_CACHE["nc"]


def kernel(x: np.ndarray, y: np.ndarray) -> np.ndarray:
    assert x.shape == (B, M, D) and y.shape == (B, N, D)
    nc = _get_nc()
    ident = np.eye(P, dtype=np.float16)
    in_maps = [
        {
            "x": np.ascontiguousarray(x[b], dtype=np.float32),
            "y": np.ascontiguousarray(y[b], dtype=np.float32),
            "ident": ident,
        }
        for b in range(B)
    ]
    res = bass_utils.run_bass_kernel_spmd(nc, in_maps, core_ids=list(range(B)))
    return np.stack([res.results[b]["out"] for b in range(B)], axis=0)
